# revision 13
# baseline (speedup 1.0000x reference)
"""HOG (histogram of oriented gradients) Bass kernel for Trainium2.

8-core data parallel: core i processes images [8i, 8i+8) of the full
[64, 1, 512, 512] input. Per image:
  - Sobel gx/gy via separable shifts+adds. Vertical neighbor access uses
    row-shifted SBUF copies XU/XD made by SBUF->SBUF DMA (compute-engine
    APs must start at quadrant-aligned partitions, DMA APs are free).
    Horizontal access uses zero-padded columns.
  - mag = sqrt(gx^2+gy^2); division-free angle binning on the monotone
    proxy u = gy/(|gx|+|gy|): bin boundary tests [ang >= k*pi/8] become
    [u <= u_k] with u_k = cos(t)/(sin(t)+|cos(t)|) at t = k*pi/8.
  - Masked magnitudes V_k = (u <= u_k) * mag are vertically 8-pooled on
    the TensorEngine into a stacked PSUM [128 = 8 bins x 16 cellrows,
    2048]. The pooling weights carry +P8/-P8 blocks so the PSUM
    accumulation directly forms hist_k = S_k - S_{k+1}; one grouped DVE
    reduce then does the horizontal 8-pool. Bin 0 uses its own mask
    (u > u_1) with a small horizontal-reduce-then-pool path.
Output: [64, 9, 64, 64] flattened, fp32.
"""

import math

import numpy as np

_IMGS = 8  # images per core
_NCORES = 8
_H = 512
_B = 4  # row blocks of 128
_P = 128

# u_k = cos(t)/(sin(t)+|cos(t)|) at t = k*pi/8, k = 1..7; k=8 uses
# -(1 - 1e-6) to catch the ang == pi (gx == 0, gy < 0) bin-8 pixels.
_UK = [
    0.7071067811865476,
    0.5,
    0.29289321881345254,
    0.0,
    -0.29289321881345254,
    -0.5,
    -0.7071067811865476,
    -0.999999,
]

_PROG = None


def _build_program():
    import concourse.bass as bass  # noqa: F401
    import concourse.mybir as mybir
    import concourse.tile as tile
    from concourse import bacc

    f32 = mybir.dt.float32
    Alu = mybir.AluOpType
    Act = mybir.ActivationFunctionType
    X_AX = mybir.AxisListType.X

    nc = bacc.Bacc("TRN2", target_bir_lowering=False, debug=False)

    x_d = nc.declare_dram_parameter("x", [_IMGS, _H, _H], f32, isOutput=False)
    # PE pool weights [128, 336]: five [128, 64] variants (w_first, w_diff1..3,
    # w_negend) that accumulate hist_k = S_k - S_{k+1} into 64-row PSUM halves
    # (matmul outputs must start at partition 0/64), then the plain [128, 16]
    # ones-pool at cols 320:336 for the bin-0 path.
    p8_d = nc.declare_dram_parameter("p8", [_P, 336], f32, isOutput=False)
    z_d = nc.declare_dram_parameter("zeros", [2048], f32, isOutput=False)
    o_d = nc.declare_dram_parameter("out", [_IMGS, 9, 64, 64], f32, isOutput=True)

    zrow = z_d[0:512].rearrange("(p c) -> p c", p=1)  # [1, 512]
    zpad = z_d[0:512].rearrange("(p b c) -> p b c", p=_P, b=_B)  # [128, 4, 1]

    with tile.TileContext(nc) as tc:
        with (
            tc.tile_pool(name="const", bufs=1) as constp,
            tc.tile_pool(name="sb", bufs=1) as sb,
            tc.tile_pool(name="ps", bufs=1, space="PSUM") as ps,
        ):
            p8 = constp.tile([_P, 336], f32, name="p8t")
            nc.sync.dma_start(p8[:], p8_d[:])
            w_var = [p8[:, 64 * m : 64 * m + 64] for m in range(5)]
            pool16 = p8[:, 320:336]

            for img in range(_IMGS):
                nm = f"i{img}"
                X = sb.tile([_P, _B, _H], f32, tag="x", bufs=2, name=f"x{nm}")
                nc.sync.dma_start(
                    X[:], x_d[img].rearrange("(b p) c -> p b c", p=_P)
                )

                # Row-shifted copies (vertical neighbor access), via DMA.
                XU = sb.tile([_P, _B, _H], f32, tag="xu", bufs=1, name=f"xu{nm}")
                nc.sync.dma_start(XU[1:128], X[0:127])
                nc.sync.dma_start(XU[0:1, 1:4, :], X[127:128, 0:3, :])
                nc.sync.dma_start(XU[0:1, 0, :], zrow)
                XD = sb.tile([_P, _B, _H], f32, tag="xd", bufs=1, name=f"xd{nm}")
                nc.sync.dma_start(XD[0:127], X[1:128])
                nc.sync.dma_start(XD[127:128, 0:3, :], X[0:1, 1:4, :])
                nc.sync.dma_start(XD[127:128, 3, :], zrow)

                # Vertical smooth SM = XU + 2X + XD and diff Dv = XD - XU,
                # written into column-padded buffers (pads zeroed via DMA).
                A = sb.tile([_P, _B, _H], f32, tag="atmp", bufs=1, name=f"a{nm}")
                nc.gpsimd.tensor_add(A[:], XU[:], XD[:])
                SM = sb.tile([_P, _B, 514], f32, tag="sm", bufs=1, name=f"sm{nm}")
                nc.sync.dma_start(SM[:, :, 0:1], zpad)
                nc.sync.dma_start(SM[:, :, 513:514], zpad)
                nc.vector.scalar_tensor_tensor(
                    SM[:, :, 1:513], X[:], 2.0, A[:], op0=Alu.mult, op1=Alu.add
                )
                Dv = sb.tile([_P, _B, 514], f32, tag="d", bufs=1, name=f"d{nm}")
                nc.sync.dma_start(Dv[:, :, 0:1], zpad)
                nc.sync.dma_start(Dv[:, :, 513:514], zpad)
                nc.gpsimd.tensor_sub(Dv[:, :, 1:513], XD[:], XU[:])

                # Horizontal diff / smooth.
                GX = sb.tile([_P, _B, _H], f32, tag="gx", bufs=1, name=f"gx{nm}")
                nc.vector.tensor_sub(GX[:], SM[:, :, 2:514], SM[:, :, 0:512])
                A2 = sb.tile([_P, _B, _H], f32, tag="atmp", bufs=1, name=f"a2{nm}")
                nc.gpsimd.tensor_add(A2[:], Dv[:, :, 0:512], Dv[:, :, 2:514])
                GY = sb.tile([_P, _B, _H], f32, tag="gy", bufs=1, name=f"gy{nm}")
                nc.vector.scalar_tensor_tensor(
                    GY[:], Dv[:, :, 1:513], 2.0, A2[:], op0=Alu.mult, op1=Alu.add
                )

                # |gx|+|gy| and magnitude.
                ABSX = sb.tile([_P, _B, _H], f32, tag="sq", bufs=2, name=f"ax{nm}")
                nc.scalar.activation(ABSX[:], GX[:], Act.Abs)
                ABSY = sb.tile([_P, _B, _H], f32, tag="sq", bufs=2, name=f"ay{nm}")
                nc.scalar.activation(ABSY[:], GY[:], Act.Abs)
                DN = sb.tile([_P, _B, _H], f32, tag="dn", bufs=1, name=f"dn{nm}")
                nc.gpsimd.tensor_add(DN[:], ABSX[:], ABSY[:])
                SQX = sb.tile([_P, _B, _H], f32, tag="sq", bufs=2, name=f"sx{nm}")
                nc.scalar.square(SQX[:], GX[:])
                SQY = sb.tile([_P, _B, _H], f32, tag="sq", bufs=2, name=f"sy{nm}")
                nc.scalar.square(SQY[:], GY[:])
                M2 = sb.tile([_P, _B, _H], f32, tag="m2", bufs=1, name=f"m2{nm}")
                nc.gpsimd.tensor_add(M2[:], SQX[:], SQY[:])
                MAG = sb.tile([_P, _B, _H], f32, tag="mag", bufs=1, name=f"mg{nm}")
                nc.scalar.sqrt(MAG[:], M2[:])

                RS = sb.tile([_P, _B, _H], f32, tag="rec", bufs=2, name=f"rs{nm}")
                R = sb.tile([_P, _B, _H], f32, tag="rec", bufs=2, name=f"r{nm}")
                nc.vector.reciprocal_approx_accurate(R[:], DN[:], RS[:])
                U = sb.tile([_P, _B, _H], f32, tag="u", bufs=1, name=f"u{nm}")
                nc.vector.tensor_mul(U[:], GY[:], R[:])

                # Stacked PSUM [128, 4, 512]: half 0 rows 16(k-1) hold
                # hist_k for k=1..4, half 1 for k=5..8, accumulated by the
                # +/-P8 weight variants.
                PST = ps.tile([_P, _B, _H], f32, tag="pst", bufs=1, name=f"pst{nm}")
                PS0 = ps.tile([16, 256], f32, tag="ps0", bufs=1, name=f"ps0{nm}")

                # (weight variant, half, start, stop) per bin j=1..8; V5
                # contributes to both halves (negend closes half 0).
                for j, uk in enumerate(_UK, start=1):
                    V = sb.tile([_P, _B, _H], f32, tag="v", bufs=3, name=f"v{nm}_{j}")
                    nc.vector.scalar_tensor_tensor(
                        V[:], U[:], float(uk), MAG[:], op0=Alu.is_le, op1=Alu.mult
                    )
                    for b in range(_B):
                        if j <= 4:
                            nc.tensor.matmul(
                                PST[0:64, b, :], w_var[j - 1], V[:, b, :],
                                start=(j == 1), stop=False,
                            )
                        if j == 5:
                            nc.tensor.matmul(
                                PST[0:64, b, :], w_var[4], V[:, b, :],
                                start=False, stop=True,
                            )
                        if j >= 5:
                            nc.tensor.matmul(
                                PST[64:128, b, :], w_var[j - 5], V[:, b, :],
                                start=(j == 5), stop=(j == 8),
                            )

                # Bin 0: V0 = mag * (u > u_1); horizontal 8-pool then PE pool.
                V0 = sb.tile([_P, _B, _H], f32, tag="v", bufs=3, name=f"v{nm}_0")
                nc.vector.scalar_tensor_tensor(
                    V0[:], U[:], _UK[0], MAG[:], op0=Alu.is_gt, op1=Alu.mult
                )
                H0R = sb.tile([_P, _B, 64], f32, tag="h0r", bufs=2, name=f"h0r{nm}")
                nc.vector.reduce_sum(
                    H0R[:], V0.rearrange("p b (cc e) -> p b cc e", e=8), axis=X_AX
                )
                nc.tensor.matmul(
                    PS0[:], pool16, H0R.rearrange("p b c -> p (b c)"),
                    start=True, stop=True,
                )

                # Horizontal 8-pool of the stacked bins -> hist_1..8.
                G = sb.tile([_P, _B, 64], f32, tag="g", bufs=2, name=f"g{nm}")
                nc.vector.reduce_sum(
                    G[:], PST.rearrange("p b (cc e) -> p b cc e", e=8), axis=X_AX
                )

                H0 = sb.tile([16, _B, 64], f32, tag="h0", bufs=2, name=f"h0{nm}")
                nc.scalar.copy(H0[:], PS0.rearrange("p (b c) -> p b c", b=_B))
                nc.sync.dma_start(
                    o_d[img, 0].rearrange("(b m) c -> m b c", m=16), H0[:]
                )
                for k in range(1, 9):
                    nc.sync.dma_start(
                        o_d[img, k].rearrange("(b m) c -> m b c", m=16),
                        G[16 * (k - 1) : 16 * k],
                    )

    nc.finalize()
    return nc


def _get_program():
    global _PROG
    if _PROG is None:
        _PROG = _build_program()
    return _PROG


def _pool_weights():
    p16 = np.repeat(np.eye(16, dtype=np.float32), 8, axis=0)  # [128, 16]
    w = np.zeros((128, 336), dtype=np.float32)
    w[:, 0:16] = p16  # w_first: +P8 @ slot 0
    for m in (1, 2, 3):  # w_diffm: +P8 @ slot m, -P8 @ slot m-1
        w[:, 64 * m + 16 * m : 64 * m + 16 * m + 16] = p16
        w[:, 64 * m + 16 * (m - 1) : 64 * m + 16 * (m - 1) + 16] = -p16
    w[:, 256 + 48 : 256 + 64] = -p16  # w_negend: -P8 @ slot 3
    w[:, 320:336] = p16
    return w


def kernel(x, sobel_x=None, sobel_y=None):
    x = np.ascontiguousarray(
        np.asarray(x, dtype=np.float32).reshape(_NCORES * _IMGS, _H, _H)
    )
    nc = _get_program()

    from concourse.bass_utils import run_bass_kernel_spmd

    p8 = _pool_weights()
    zeros = np.zeros(2048, dtype=np.float32)
    in_maps = [
        {"x": x[_IMGS * i : _IMGS * (i + 1)], "p8": p8, "zeros": zeros}
        for i in range(_NCORES)
    ]
    res = run_bass_kernel_spmd(nc, in_maps, list(range(_NCORES)))
    out = np.concatenate(
        [np.asarray(r["out"]) for r in res.results], axis=0
    )  # [64, 9, 64, 64]
    return out.reshape(-1).astype(np.float32)
